# revision 7
# baseline (speedup 1.0000x reference)
"""AQT int8-quantized matmul (dynamic symmetric quantization) on 8 TRN2 cores.

Full problem: lhs [8192, 4096] f32 @ rhs [4096, 4096] f32 with per-row lhs
scales and per-column rhs scales (abs-max / 127.5), int8 round+clip, int32
matmul, dequantize by the outer product of scales.

Sharding: 2x4 grid over (M, N). Each core gets lhs rows M/2 and rhs cols N/4,
computes its [4096, 1024] output block; host assembles the 8 blocks. Both
quantization axes keep their full contraction dim on every core, so per-core
results match the unsharded reference exactly. No collectives needed.

Per-core kernel: quantized values are exact integers in [-127, 127] stored as
bf16; TensorE matmul with fp32 PSUM accumulation reproduces the int32 matmul.
round() is exact via the +1.5*2^23 magic-constant trick; the quant divisor is
shrunk by (1-2^-20) so no post-round clip is needed (matches reference
round-then-clip on abs-max elements).

Scheduling (v2, stall-focused rewrite of the 694us baseline):
- Emission interleaves rhs quantization with lhs quantization and matmuls so
  TensorE starts ~25us in instead of 81us. rhs nb=0 is quantized first; the
  first DUAL_AT m-tiles run nb0-only matmul chains (kt-lockstep across two
  m-tiles while quantized rhs tiles trickle in at DMA pace); nb=1 streams in
  under those matmuls; m-tiles >= DUAL_AT run both chains; the nb1 chains of
  the first DUAL_AT m-tiles run at the end (their lhsT tiles stay pinned).
- DMA queue split: lhs/rhs/out transfers on the SP HWDGE queue; the lhs
  DMA-xbar transposes (which BLOCK their queue ~1.3us per chunk) are issued
  from the Activation HWDGE queue, chunked 4x[128,1024] to pipeline.
- Engine split: ACT = lhs scale+round (chunked) + rhs Abs; DVE = reductions,
  rhs max/quant, fused psum eviction scalar_tensor_tensor((psum*s_l)*s_bc);
  GpSimd = partition_all_reduce only.
"""
import sys

if "/opt/trn_rl_repo" not in sys.path:
    sys.path.insert(0, "/opt/trn_rl_repo")

from contextlib import ExitStack

import numpy as np

from concourse import bacc, bass_isa, mybir, tile
from concourse.bass_utils import run_bass_kernel_spmd

f32 = mybir.dt.float32
bf16 = mybir.dt.bfloat16
Alu = mybir.AluOpType
Act = mybir.ActivationFunctionType

P = 128
C_MAGIC = 1.5 * 2 ** 23
QDIV = 127.5 * (1.0 - 2.0 ** -20)
INV_QDIV = 1.0 / QDIV
TINY = 1e-30

M, K, N = 8192, 4096, 4096
MG, NG = 2, 4                      # shard grid rows (M) x cols (N)
M_loc, N_loc = M // MG, N // NG    # 4096, 1024 per core
N_CORES = MG * NG

CHK = 1024                         # lhs quant chunk (free-dim elems)
DUAL_AT = 3                        # first m-tile running both nb chains


def build_aqt(nc, M_loc, K, N_loc, W=512):
    KT, MT, NB = K // P, M_loc // P, N_loc // W
    NCHK = K // CHK
    KPC = CHK // P                 # k-tiles per chunk (8)

    lhs = nc.declare_dram_parameter("lhs", [M_loc, K], f32, isOutput=False)
    rhs = nc.declare_dram_parameter("rhs", [K, N_loc], f32, isOutput=False)
    out = nc.declare_dram_parameter("out", [M_loc, N_loc], f32, isOutput=True)

    with tile.TileContext(nc) as tc, ExitStack() as ctx:
        pool = lambda name, bufs, **kw: ctx.enter_context(
            tc.tile_pool(name=name, bufs=bufs, **kw))
        qr_pool = pool("qr", NB * KT)       # quantized rhs, resident (64KB/p)
        sbc_pool = pool("sbc", NB)          # rhs dequant scales, resident
        rstage = pool("rstage", 3)          # rhs raw pass A
        rstage2 = pool("rstage2", 3)        # rhs raw pass B
        rmul = pool("rmul", 2)              # |rhs| / rhs * r_bc
        racc = pool("racc", 2)              # absmax accumulator ping-pong
        rbc = pool("rbc", 2)                # amax_bc / r_bc
        lraw = pool("lraw", 2)              # lhs raw [P, K] f32 (32KB/p)
        lt1 = pool("lt1", 3)                # lhs scaled+C chunk [P, CHK] f32
        lqc = pool("lqc", 3)                # lhs quantized chunk [P, CHK] bf16
        lqt_pin = pool("lqt_pin", DUAL_AT)  # lhsT tiles pinned for tail chains
        lqt_rot = pool("lqt_rot", 3)        # lhsT tiles, rotating
        lsc = pool("lsc", 1)                # s_l columns, resident
        lam = pool("lam", 4)                # [P, 1] scratch
        opool = pool("o1", 3)
        psum = ctx.enter_context(tc.tile_pool(name="psum", bufs=8, space="PSUM"))

        s_l_all = lsc.tile([P, MT], f32)

        raw_tiles = {}
        rl_tiles = {}
        qt_tiles = {}
        qr_tiles = {}
        sbc_tiles = {}
        racc_state = {}

        # ---------------- lhs helpers ----------------
        def lhs_load(mi):
            raw = lraw.tile([P, K], f32, name="lraw")
            nc.sync.dma_start(raw[:], lhs[mi * P:(mi + 1) * P, :])
            raw_tiles[mi] = raw

        def lhs_reduce(mi):
            raw = raw_tiles[mi]
            am = lam.tile([P, 1], f32, name="lam")
            nc.vector.tensor_reduce(am[:], raw[:], axis=mybir.AxisListType.X,
                                    op=Alu.max, apply_absolute_value=True)
            s_col = s_l_all[:, mi:mi + 1]
            nc.vector.tensor_scalar(s_col, am[:], TINY, INV_QDIV,
                                    op0=Alu.max, op1=Alu.mult)
            r_l = lam.tile([P, 1], f32, name="rl")
            nc.vector.reciprocal(r_l[:], s_col)
            rl_tiles[mi] = r_l

        def lhs_chunk(mi, c):
            # quantize chunk c and transpose it into qt[:, c*KPC:(c+1)*KPC, :]
            raw = raw_tiles[mi]
            if c == 0:
                qt = (lqt_pin if mi < DUAL_AT else lqt_rot).tile(
                    [P, KT, P], bf16, name="lqt")
                qt_tiles[mi] = qt
            qt = qt_tiles[mi]
            cs = slice(c * CHK, (c + 1) * CHK)
            t1 = lt1.tile([P, CHK], f32, name="lt1")
            nc.scalar.activation(t1[:], raw[:, cs], Act.Copy,
                                 bias=C_MAGIC, scale=rl_tiles[mi][:])
            qc = lqc.tile([P, CHK], bf16, name="lqc")
            nc.scalar.activation(qc[:], t1[:], Act.Copy, bias=-C_MAGIC)
            nc.scalar.dma_start_transpose(
                qt[:, c * KPC:(c + 1) * KPC, :], qc[:])
            if c == NCHK - 1:
                del raw_tiles[mi]

        def lhs_quant(mi):
            lhs_reduce(mi)
            for c in range(NCHK):
                lhs_chunk(mi, c)

        # ---------------- rhs helpers ----------------
        def rhs_A(nb, kt):
            cs = slice(nb * W, (nb + 1) * W)
            t = rstage.tile([P, W], f32, name="rstage")
            nc.sync.dma_start(t[:], rhs[kt * P:(kt + 1) * P, cs])
            ta = rmul.tile([P, W], f32, name="rabs")
            nc.scalar.activation(ta[:], t[:], Act.Abs)
            acc = racc_state.get(nb)
            nacc = racc.tile([P, W], f32, name="racc")
            nc.vector.tensor_tensor(nacc[:], (acc or ta)[:], ta[:], op=Alu.max)
            racc_state[nb] = nacc

        def rhs_scales(nb):
            amax = rbc.tile([P, W], f32, name="amax")
            nc.gpsimd.partition_all_reduce(amax[:], racc_state[nb][:],
                                           channels=P,
                                           reduce_op=bass_isa.ReduceOp.absmax)
            s_bc = sbc_pool.tile([P, W], f32, name="sbc")
            nc.vector.tensor_scalar(s_bc[:], amax[:], TINY, INV_QDIV,
                                    op0=Alu.max, op1=Alu.mult)
            sbc_tiles[nb] = s_bc
            r_bc = rbc.tile([P, W], f32, name="rbc")
            nc.vector.reciprocal(r_bc[:], s_bc[:])
            return r_bc

        def rhs_B(nb, kt, r_bc):
            cs = slice(nb * W, (nb + 1) * W)
            t2 = rstage2.tile([P, W], f32, name="rstage2")
            nc.sync.dma_start(t2[:], rhs[kt * P:(kt + 1) * P, cs])
            u = rmul.tile([P, W], f32, name="rmul")
            nc.vector.tensor_tensor(u[:], t2[:], r_bc[:], op=Alu.mult)
            q = qr_pool.tile([P, W], bf16, name="qr")
            nc.vector.tensor_scalar(q[:], u[:], C_MAGIC, C_MAGIC,
                                    op0=Alu.add, op1=Alu.subtract)
            qr_tiles[(nb, kt)] = q

        # ---------------- matmul + eviction ----------------
        def evict(mi, nb, ps):
            o = opool.tile([P, W], f32, name="o1")
            nc.vector.scalar_tensor_tensor(
                o[:], ps[:], s_l_all[:, mi:mi + 1], sbc_tiles[nb][:],
                op0=Alu.mult, op1=Alu.mult)
            nc.sync.dma_start(
                out[mi * P:(mi + 1) * P, nb * W:(nb + 1) * W], o[:])

        def chain(mi, nb):
            ps = psum.tile([P, W], f32, name="ps")
            qt = qt_tiles[mi]
            for kt in range(KT):
                nc.tensor.matmul(ps[:], qt[:, kt, :], qr_tiles[(nb, kt)][:],
                                 start=(kt == 0), stop=(kt == KT - 1))
            evict(mi, nb, ps)

        def lockstep(pairs):
            # pairs of (mi, nb) advanced one k-tile at a time so a chain paced
            # by trickling rhs DMAs doesn't leave TensorE idle
            pss = {pr: psum.tile([P, W], f32, name="ps") for pr in pairs}
            for kt in range(KT):
                for mi, nb in pairs:
                    nc.tensor.matmul(pss[(mi, nb)][:], qt_tiles[mi][:, kt, :],
                                     qr_tiles[(nb, kt)][:],
                                     start=(kt == 0), stop=(kt == KT - 1))
            for mi, nb in pairs:
                evict(mi, nb, pss[(mi, nb)])

        # ---------------- emission script ----------------
        # prologue: lhs 0/1 load + rhs nb0 pass A, interleaved
        lhs_load(0)
        lhs_load(1)
        for kt in range(KT):
            rhs_A(0, kt)
            if kt == 10:
                lhs_reduce(0)
            elif 12 <= kt < 12 + NCHK:
                lhs_chunk(0, kt - 12)
        r_bc0 = rhs_scales(0)
        lhs_quant(1)
        for kt in range(KT):
            rhs_B(0, kt, r_bc0)
        lhs_load(2)

        # matmuls start (paced by rhs_B(0) DMAs); nb1 pass A streams under them
        lockstep([(0, 0), (1, 0)])
        for kt in range(KT):
            rhs_A(1, kt)
        lhs_quant(2)
        lhs_load(3)
        chain(2, 0)
        r_bc1 = rhs_scales(1)
        lhs_quant(3)
        lhs_load(4)
        for kt in range(KT):
            rhs_B(1, kt, r_bc1)
        chain(3, 0)
        lhs_quant(4)
        lhs_load(5)
        # first nb1 chains are paced by rhs_B(1) DMAs: lockstep them with
        # resident-nb0 work to keep TensorE fed
        lockstep([(3, 1), (4, 0), (4, 1)])

        # steady state: both chains per m-tile
        for mi in range(5, MT):
            lhs_quant(mi)
            if mi + 1 < MT:
                lhs_load(mi + 1)
            chain(mi, 0)
            chain(mi, 1)

        # tail: nb1 chains of the pinned early m-tiles
        for mi in range(DUAL_AT):
            chain(mi, 1)
    return nc


_COMPILED_NC = None


def _get_compiled():
    global _COMPILED_NC
    if _COMPILED_NC is None:
        nc = bacc.Bacc("TRN2", target_bir_lowering=False, debug=False,
                       num_devices=N_CORES)
        build_aqt(nc, M_loc, K, N_loc)
        nc.compile()
        _COMPILED_NC = nc
    return _COMPILED_NC


def _shard(lhs, rhs):
    in_maps = []
    for i in range(N_CORES):
        mg, ng = divmod(i, NG)
        in_maps.append({
            "lhs": np.ascontiguousarray(lhs[mg * M_loc:(mg + 1) * M_loc, :]),
            "rhs": np.ascontiguousarray(rhs[:, ng * N_loc:(ng + 1) * N_loc]),
        })
    return in_maps


def kernel(lhs, rhs, _trace=False, _trace_kwargs=None):
    lhs = np.asarray(lhs, np.float32)
    rhs = np.asarray(rhs, np.float32)
    nc = _get_compiled()
    res = run_bass_kernel_spmd(nc, _shard(lhs, rhs), core_ids=list(range(N_CORES)),
                               trace=_trace, **(_trace_kwargs or {}))
    out = np.empty((M, N), np.float32)
    for i in range(N_CORES):
        mg, ng = divmod(i, NG)
        out[mg * M_loc:(mg + 1) * M_loc, ng * N_loc:(ng + 1) * N_loc] = \
            res.results[i]["out"]
    kernel.last_result = res
    return out
